# revision 8
# baseline (speedup 1.0000x reference)
"""EngramCache forward kernel for 8x Trainium2 NeuronCores (Bass/Tile).

Strategy: data-parallel over the 16384 tokens (2048 per core), hash tables
replicated. Host-side prep: n-gram hash indices (computed with jax-on-CPU to
bit-match the reference's int32 % lowering), tables merged to one
[8*TS, 256] fp16 table, Wv transposed to k-major fp16. Per core: one
indirect-DMA gather per 128-token tile -> PE transpose to k-major ->
fp16 matmul (fp32 PSUM accumulate) against SBUF-resident WvT -> fused
RMSNorm-gating epilogue on ACT/DVE.
"""

import numpy as np


def _ensure_import_paths():
    import sys
    for p in ("/opt/trn_rl_repo", "/root/.axon_site/_ro/trn_rl_repo"):
        if p not in sys.path:
            sys.path.insert(0, p)


_ensure_import_paths()

B, T, D, H, E = 4, 4096, 2048, 4, 256
TS, VOCAB = 100000, 50257
M0, M1, M2 = 20011, 30011, 40009
NCORES = 8
NT = B * T                 # 16384 tokens total
TPC = NT // NCORES         # 2048 tokens per core
P = 128
NTILES = TPC // P          # 16 token-tiles per core
HE = H * E                 # 1024: one table's concat-over-heads row
KCH = D // P               # 16 k-chunks of 128
DCH = 512                  # matmul moving free dim (one PSUM bank)
EPS = float(np.finfo(np.float32).eps)

_PROGRAM = None


def _host_indices(input_ids: np.ndarray) -> np.ndarray:
    """Merged-table row index per (token, segment): [NT, SEG] int32.

    Uses jax on CPU so the int32 `%` matches the reference bit-exactly
    (XLA-CPU lowers int32 division through a float path that yields
    negative remainders near the modulus; the reference clamps with max).
    """
    import jax
    import jax.numpy as jnp
    with jax.default_device(jax.devices("cpu")[0]):
        ids = jnp.clip(jnp.asarray(input_ids).astype(jnp.int32), 0, VOCAB - 1)
        s1 = jnp.pad(ids[:, :-1], ((0, 0), (1, 0)))
        s2 = jnp.pad(ids[:, :-2], ((0, 0), (2, 0)))
        hash2 = jnp.bitwise_xor(ids * M0, s1 * M1)
        hash3 = jnp.bitwise_xor(hash2, s2 * M2)
        i2 = np.asarray(jnp.maximum(hash2 % TS, 0)).reshape(-1)
        i3 = np.asarray(jnp.maximum(hash3 % TS, 0)).reshape(-1)
    idx = np.empty((NT, 2), np.int32)
    idx[:, 0] = i2
    idx[:, 1] = i3
    return idx


def _build_program():
    import concourse.bass as bass
    import concourse.bacc as bacc
    import concourse.mybir as mybir
    import concourse.tile as tile

    dt = mybir.dt
    nc = bacc.Bacc("TRN2", target_bir_lowering=False, debug=False)

    tab2 = nc.dram_tensor("tab2", [TS, HE], dt.float16, kind="ExternalInput")
    tab3 = nc.dram_tensor("tab3", [TS, HE], dt.float16, kind="ExternalInput")
    wvt = nc.dram_tensor("wvt", [D, D], dt.float16, kind="ExternalInput")
    wprod = nc.dram_tensor("wprod", [P, D], dt.float32, kind="ExternalInput")
    idx = nc.dram_tensor("idx", [P, NTILES * 2], dt.int32, kind="ExternalInput")
    hid = nc.dram_tensor("hid", [TPC, D], dt.float32, kind="ExternalInput")
    out = nc.dram_tensor("out", [TPC, D], dt.float32, kind="ExternalOutput")

    AF = mybir.ActivationFunctionType
    OP = mybir.AluOpType

    with tile.TileContext(nc) as tc:
        with (
            tc.tile_pool(name="persist", bufs=1) as persist,
            tc.tile_pool(name="eP", bufs=3) as e_pool,
            tc.tile_pool(name="etP", bufs=2) as et_pool,
            tc.tile_pool(name="hP", bufs=3) as h_pool,
            tc.tile_pool(name="oP", bufs=2) as o_pool,
            tc.tile_pool(name="wP", bufs=2) as vw_pool,
            tc.tile_pool(name="stats", bufs=2) as st_pool,
            tc.tile_pool(name="psV", bufs=2, space="PSUM") as psV,
        ):
            # ---- one-time loads ----
            # small tensors first so tile 0's gather/transpose aren't gated
            # behind the 16 MiB WvT stream
            idx_sb = persist.tile([P, NTILES * 2], dt.int32)
            nc.sync.dma_start(out=idx_sb[:], in_=idx[:])
            wprod_sb = persist.tile([P, D], dt.float32)
            nc.sync.dma_start(out=wprod_sb[:], in_=wprod[:])

            def issue_loads(i):
                """Gather + hidden load + xbar transpose for tile i."""
                e_sb = e_pool.tile([P, 2 * HE], dt.float16, tag="e")
                nc.gpsimd.indirect_dma_start(
                    out=e_sb[:, 0:HE],
                    out_offset=None,
                    in_=tab2[:],
                    in_offset=bass.IndirectOffsetOnAxis(
                        ap=idx_sb[:, 2 * i:2 * i + 1], axis=0
                    ),
                )
                nc.gpsimd.indirect_dma_start(
                    out=e_sb[:, HE:2 * HE],
                    out_offset=None,
                    in_=tab3[:],
                    in_offset=bass.IndirectOffsetOnAxis(
                        ap=idx_sb[:, 2 * i + 1:2 * i + 2], axis=0
                    ),
                )
                h_sb = h_pool.tile([P, D], dt.float32, tag="h")
                nc.sync.dma_start(out=h_sb[:], in_=hid[i * P:(i + 1) * P, :])
                # out[p, c, t] = e[t, c*128 + p]: chunk-major k layout
                eT_sb = et_pool.tile([P, D], dt.float16, tag="eT")
                nc.sync.dma_start_transpose(
                    out=eT_sb[:].rearrange("p (c t) -> p c t", c=KCH),
                    in_=e_sb[:],
                )
                return h_sb, eT_sb

            # prefetch tiles 0/1 ahead of the WvT stream
            pending = [issue_loads(0), issue_loads(1)]

            # WvT split into per-k-chunk DMAs so matmul k=0 starts after the
            # first chunk lands instead of after the whole 8 MiB
            wvt_sb = persist.tile([P, KCH * D], dt.float16)   # 64KB/partition
            wvt_r = wvt[:].rearrange("(c p) d -> c p d", p=P)
            for c in range(KCH):
                nc.sync.dma_start(
                    out=wvt_sb[:, c * D:(c + 1) * D], in_=wvt_r[c]
                )

            for i in range(NTILES):
                h_sb, eT_sb = pending[i]
                if i + 2 < NTILES:
                    pending.append(issue_loads(i + 2))

                # ---- matmul: v[t, d] = sum_k e[t, k] * WvT[k, d] ----
                v_ps = psV.tile([P, D], dt.float32, tag="vps")
                for k in range(KCH):
                    lhsT = eT_sb[:, k * P:(k + 1) * P]
                    for dc in range(D // DCH):
                        nc.tensor.matmul(
                            v_ps[:, dc * DCH:(dc + 1) * DCH],
                            lhsT,
                            wvt_sb[:, k * D + dc * DCH: k * D + (dc + 1) * DCH],
                            start=(k == 0),
                            stop=(k == KCH - 1),
                        )

                # ---- epilogue (reads v straight from PSUM; psV bufs=2
                # lets the next tile's matmuls proceed meanwhile) ----
                vw_sb = vw_pool.tile([P, D], dt.float32, tag="vw")
                out_sb = o_pool.tile([P, D], dt.float32, tag="o")

                sh = st_pool.tile([P, 1], dt.float32, tag="sh")
                sv = st_pool.tile([P, 1], dt.float32, tag="sv")
                gd = st_pool.tile([P, 1], dt.float32, tag="gd")

                # sum h^2 (junk full output into vw_sb, overwritten below)
                nc.scalar.activation(vw_sb[:], h_sb[:], AF.Square, accum_out=sh[:])
                # sum v^2 (junk full output into out_sb, overwritten below)
                nc.scalar.activation(out_sb[:], v_ps[:], AF.Square, accum_out=sv[:])
                # vw = v * wprod
                nc.vector.tensor_tensor(
                    out=vw_sb[:], in0=v_ps[:], in1=wprod_sb[:], op=OP.mult
                )
                # gd = sum(vw * h)
                nc.vector.scalar_tensor_tensor(
                    out=out_sb[:], in0=vw_sb[:], scalar=1.0, in1=h_sb[:],
                    op0=OP.mult, op1=OP.mult, accum_out=gd[:],
                )

                a_ = st_pool.tile([P, 1], dt.float32, tag="a")
                b_ = st_pool.tile([P, 1], dt.float32, tag="b")
                r_ = st_pool.tile([P, 1], dt.float32, tag="r")
                rr = st_pool.tile([P, 1], dt.float32, tag="rr")
                s_ = st_pool.tile([P, 1], dt.float32, tag="s")
                gr = st_pool.tile([P, 1], dt.float32, tag="gr")
                ab = st_pool.tile([P, 1], dt.float32, tag="ab")
                pq = st_pool.tile([P, 1], dt.float32, tag="pq")
                g1 = st_pool.tile([P, 1], dt.float32, tag="g1")
                g2 = st_pool.tile([P, 1], dt.float32, tag="g2")
                sg = st_pool.tile([P, 1], dt.float32, tag="sg")
                pg = st_pool.tile([P, 1], dt.float32, tag="pg")
                gate = st_pool.tile([P, 1], dt.float32, tag="gate")

                # a = sh/D + eps ; b = sv/D + eps ; r = a*b ; s = rsqrt(r)
                nc.vector.tensor_scalar(a_[:], sh[:], 1.0 / D, EPS, OP.mult, OP.add)
                nc.vector.tensor_scalar(b_[:], sv[:], 1.0 / D, EPS, OP.mult, OP.add)
                nc.vector.tensor_tensor(out=r_[:], in0=a_[:], in1=b_[:], op=OP.mult)
                nc.vector.reciprocal(rr[:], r_[:])
                nc.scalar.activation(s_[:], rr[:], AF.Sqrt)
                # graw = gd * (1/sqrt(D)) * s
                nc.vector.scalar_tensor_tensor(
                    out=gr[:], in0=gd[:], scalar=float(1.0 / np.sqrt(D)), in1=s_[:],
                    op0=OP.mult, op1=OP.mult,
                )
                # signed sqrt: pg = gr * rsqrt(max(|gr|, 1e-6))
                #   == sign(gr)*sqrt(|gr|) for |gr| >= 1e-6
                nc.vector.tensor_scalar(g1[:], gr[:], -1.0, None, OP.mult)
                nc.vector.tensor_tensor(out=g2[:], in0=gr[:], in1=g1[:], op=OP.max)
                nc.vector.tensor_scalar(ab[:], g2[:], 1e-6, None, OP.max)
                nc.vector.reciprocal(sg[:], ab[:])
                nc.scalar.activation(pq[:], sg[:], AF.Sqrt)
                nc.vector.tensor_tensor(out=pg[:], in0=gr[:], in1=pq[:], op=OP.mult)
                nc.scalar.activation(gate[:], pg[:], AF.Sigmoid)

                # out = gate * v
                nc.scalar.activation(
                    out_sb[:], v_ps[:], AF.Copy, scale=gate[:]
                )
                nc.sync.dma_start(out=out[i * P:(i + 1) * P, :], in_=out_sb[:])

    nc.compile()
    return nc


def _get_program():
    global _PROGRAM
    if _PROGRAM is None:
        _PROGRAM = _build_program()
    return _PROGRAM


def _prep_inputs(hidden, tables2, tables3, Wv, gate_h_w, gate_v_w, input_ids):
    t2 = np.ascontiguousarray(
        np.asarray(tables2).transpose(1, 0, 2).reshape(TS, HE).astype(np.float16)
    )
    t3 = np.ascontiguousarray(
        np.asarray(tables3).transpose(1, 0, 2).reshape(TS, HE).astype(np.float16)
    )
    wvt16 = np.ascontiguousarray(np.asarray(Wv).T.astype(np.float16))
    wp = np.ascontiguousarray(
        np.broadcast_to(
            (np.asarray(gate_h_w) * np.asarray(gate_v_w)).astype(np.float32), (P, D)
        )
    )
    idx_full = _host_indices(np.asarray(input_ids))
    hid_full = np.asarray(hidden, dtype=np.float32).reshape(NT, D)

    in_maps = []
    for c in range(NCORES):
        sl = slice(c * TPC, (c + 1) * TPC)
        idx_c = np.ascontiguousarray(
            idx_full[sl].reshape(NTILES, P, 2).transpose(1, 0, 2).reshape(P, NTILES * 2)
        )
        in_maps.append({
            "tab2": t2,
            "tab3": t3,
            "wvt": wvt16,
            "wprod": wp,
            "idx": idx_c,
            "hid": np.ascontiguousarray(hid_full[sl]),
        })
    return in_maps


def kernel(hidden, tables2, tables3, Wv, gate_h_w, gate_v_w, input_ids, _trace=False):
    from concourse.bass_utils import run_bass_kernel_spmd

    nc = _get_program()
    in_maps = _prep_inputs(
        hidden, tables2, tables3, Wv, gate_h_w, gate_v_w, input_ids
    )
    res = run_bass_kernel_spmd(nc, in_maps, list(range(NCORES)), trace=_trace)
    out = np.concatenate([r["out"] for r in res.results], axis=0)
    kernel.last_results = res
    return out.reshape(B, T, D).astype(np.float32)


# revision 9
# speedup vs baseline: 1.1157x; 1.1157x over previous
"""EngramCache forward kernel for 8x Trainium2 NeuronCores (Bass/Tile).

Strategy: data-parallel over the 16384 tokens (2048 per core), hash tables
replicated. Host-side prep: n-gram hash indices (computed with jax-on-CPU to
bit-match the reference's int32 % lowering), tables merged to one
[8*TS, 256] fp16 table, Wv transposed to k-major fp16. Per core: one
indirect-DMA gather per 128-token tile -> PE transpose to k-major ->
fp16 matmul (fp32 PSUM accumulate) against SBUF-resident WvT -> fused
RMSNorm-gating epilogue on ACT/DVE.
"""

import numpy as np


def _ensure_import_paths():
    import sys
    for p in ("/opt/trn_rl_repo", "/root/.axon_site/_ro/trn_rl_repo"):
        if p not in sys.path:
            sys.path.insert(0, p)


_ensure_import_paths()

B, T, D, H, E = 4, 4096, 2048, 4, 256
TS, VOCAB = 100000, 50257
M0, M1, M2 = 20011, 30011, 40009
NCORES = 8
NT = B * T                 # 16384 tokens total
TPC = NT // NCORES         # 2048 tokens per core
P = 128
NTILES = TPC // P          # 16 token-tiles per core
HE = H * E                 # 1024: one table's concat-over-heads row
KCH = D // P               # 16 k-chunks of 128
DCH = 512                  # matmul moving free dim (one PSUM bank)
EPS = float(np.finfo(np.float32).eps)

_PROGRAM = None


def _host_indices(input_ids: np.ndarray) -> np.ndarray:
    """Merged-table row index per (token, segment): [NT, SEG] int32.

    Uses jax on CPU so the int32 `%` matches the reference bit-exactly
    (XLA-CPU lowers int32 division through a float path that yields
    negative remainders near the modulus; the reference clamps with max).
    """
    import jax
    import jax.numpy as jnp
    with jax.default_device(jax.devices("cpu")[0]):
        ids = jnp.clip(jnp.asarray(input_ids).astype(jnp.int32), 0, VOCAB - 1)
        s1 = jnp.pad(ids[:, :-1], ((0, 0), (1, 0)))
        s2 = jnp.pad(ids[:, :-2], ((0, 0), (2, 0)))
        hash2 = jnp.bitwise_xor(ids * M0, s1 * M1)
        hash3 = jnp.bitwise_xor(hash2, s2 * M2)
        i2 = np.asarray(jnp.maximum(hash2 % TS, 0)).reshape(-1)
        i3 = np.asarray(jnp.maximum(hash3 % TS, 0)).reshape(-1)
    idx = np.empty((NT, 2), np.int32)
    idx[:, 0] = i2
    idx[:, 1] = i3
    return idx


def _build_program():
    import concourse.bass as bass
    import concourse.bacc as bacc
    import concourse.mybir as mybir
    import concourse.tile as tile

    dt = mybir.dt
    nc = bacc.Bacc("TRN2", target_bir_lowering=False, debug=False)

    tab2 = nc.dram_tensor("tab2", [TS, HE], dt.float16, kind="ExternalInput")
    tab3 = nc.dram_tensor("tab3", [TS, HE], dt.float16, kind="ExternalInput")
    wvt = nc.dram_tensor("wvt", [D, D], dt.float16, kind="ExternalInput")
    wprod = nc.dram_tensor("wprod", [P, D], dt.float32, kind="ExternalInput")
    idx = nc.dram_tensor("idx", [P, NTILES * 2], dt.int32, kind="ExternalInput")
    hid = nc.dram_tensor("hid", [TPC, D], dt.float32, kind="ExternalInput")
    out = nc.dram_tensor("out", [TPC, D], dt.float32, kind="ExternalOutput")

    AF = mybir.ActivationFunctionType
    OP = mybir.AluOpType

    with tile.TileContext(nc) as tc:
        with (
            tc.tile_pool(name="persist", bufs=1) as persist,
            tc.tile_pool(name="eP", bufs=3) as e_pool,
            tc.tile_pool(name="etP", bufs=2) as et_pool,
            tc.tile_pool(name="hP", bufs=3) as h_pool,
            tc.tile_pool(name="oP", bufs=2) as o_pool,
            tc.tile_pool(name="vP", bufs=2) as v_pool,
            tc.tile_pool(name="wP", bufs=2) as vw_pool,
            tc.tile_pool(name="stats", bufs=2) as st_pool,
            tc.tile_pool(name="psV", bufs=2, space="PSUM") as psV,
        ):
            # ---- one-time loads ----
            # small tensors first so tile 0's gather/transpose aren't gated
            # behind the 16 MiB WvT stream
            idx_sb = persist.tile([P, NTILES * 2], dt.int32)
            nc.sync.dma_start(out=idx_sb[:], in_=idx[:])
            wprod_sb = persist.tile([P, D], dt.float32)
            nc.sync.dma_start(out=wprod_sb[:], in_=wprod[:])

            from concourse.tile_rust import add_dep_helper

            def issue_loads(i, after=None):
                """Gather + hidden load + xbar transpose for tile i."""
                e_sb = e_pool.tile([P, 2 * HE], dt.float16, tag="e")
                g1i = nc.gpsimd.indirect_dma_start(
                    out=e_sb[:, 0:HE],
                    out_offset=None,
                    in_=tab2[:],
                    in_offset=bass.IndirectOffsetOnAxis(
                        ap=idx_sb[:, 2 * i:2 * i + 1], axis=0
                    ),
                )
                g2i = nc.gpsimd.indirect_dma_start(
                    out=e_sb[:, HE:2 * HE],
                    out_offset=None,
                    in_=tab3[:],
                    in_offset=bass.IndirectOffsetOnAxis(
                        ap=idx_sb[:, 2 * i + 1:2 * i + 2], axis=0
                    ),
                )
                if after is not None:
                    # keep startup DMA bandwidth for the WvT stream
                    add_dep_helper(g1i.ins, after.ins,
                                   reason="throttle gather behind wvt")
                    add_dep_helper(g2i.ins, after.ins,
                                   reason="throttle gather behind wvt")
                h_sb = h_pool.tile([P, D], dt.float32, tag="h")
                nc.sync.dma_start(out=h_sb[:], in_=hid[i * P:(i + 1) * P, :])
                # out[p, c, t] = e[t, c*128 + p]: chunk-major k layout
                eT_sb = et_pool.tile([P, D], dt.float16, tag="eT")
                nc.sync.dma_start_transpose(
                    out=eT_sb[:].rearrange("p (c t) -> p c t", c=KCH),
                    in_=e_sb[:],
                )
                return h_sb, eT_sb

            # prefetch tiles 0/1 ahead of the WvT stream
            pending = [issue_loads(0), issue_loads(1)]

            # WvT split into per-k-chunk DMAs so matmul k=0 starts after the
            # first chunk lands instead of after the whole 8 MiB
            wvt_sb = persist.tile([P, KCH * D], dt.float16)   # 64KB/partition
            wvt_r = wvt[:].rearrange("(c p) d -> c p d", p=P)
            for c in range(KCH):
                last_wvt = nc.sync.dma_start(
                    out=wvt_sb[:, c * D:(c + 1) * D], in_=wvt_r[c]
                )

            def do_matmul(eT_sb, v_ps, k):
                lhsT = eT_sb[:, k * P:(k + 1) * P]
                for dc in range(D // DCH):
                    nc.tensor.matmul(
                        v_ps[:, dc * DCH:(dc + 1) * DCH],
                        lhsT,
                        wvt_sb[:, k * D + dc * DCH: k * D + (dc + 1) * DCH],
                        start=(k == 0),
                        stop=(k == KCH - 1),
                    )

            def epilogue(i, h_sb, v_ps):
                # copy v out of PSUM first so the next tile's matmuls can
                # claim the bank immediately
                v_sb = v_pool.tile([P, D], dt.float32, tag="v")
                nc.scalar.copy(v_sb[:], v_ps[:])
                vw_sb = vw_pool.tile([P, D], dt.float32, tag="vw")
                out_sb = o_pool.tile([P, D], dt.float32, tag="o")

                sh = st_pool.tile([P, 1], dt.float32, tag="sh")
                sv = st_pool.tile([P, 1], dt.float32, tag="sv")
                gd = st_pool.tile([P, 1], dt.float32, tag="gd")

                # sum h^2 (junk full output into vw_sb, overwritten below)
                nc.scalar.activation(vw_sb[:], h_sb[:], AF.Square, accum_out=sh[:])
                # sum v^2 (junk full output into out_sb, overwritten below)
                nc.scalar.activation(out_sb[:], v_sb[:], AF.Square, accum_out=sv[:])
                # vw = v * wprod
                nc.vector.tensor_tensor(
                    out=vw_sb[:], in0=v_sb[:], in1=wprod_sb[:], op=OP.mult
                )
                # gd = sum(vw * h)
                nc.vector.scalar_tensor_tensor(
                    out=out_sb[:], in0=vw_sb[:], scalar=1.0, in1=h_sb[:],
                    op0=OP.mult, op1=OP.mult, accum_out=gd[:],
                )

                a_ = st_pool.tile([P, 1], dt.float32, tag="a")
                b_ = st_pool.tile([P, 1], dt.float32, tag="b")
                r_ = st_pool.tile([P, 1], dt.float32, tag="r")
                rr = st_pool.tile([P, 1], dt.float32, tag="rr")
                s_ = st_pool.tile([P, 1], dt.float32, tag="s")
                gr = st_pool.tile([P, 1], dt.float32, tag="gr")
                ab = st_pool.tile([P, 1], dt.float32, tag="ab")
                pq = st_pool.tile([P, 1], dt.float32, tag="pq")
                g1 = st_pool.tile([P, 1], dt.float32, tag="g1")
                g2 = st_pool.tile([P, 1], dt.float32, tag="g2")
                sg = st_pool.tile([P, 1], dt.float32, tag="sg")
                pg = st_pool.tile([P, 1], dt.float32, tag="pg")
                gate = st_pool.tile([P, 1], dt.float32, tag="gate")

                # a = sh/D + eps ; b = sv/D + eps ; r = a*b ; s = rsqrt(r)
                nc.vector.tensor_scalar(a_[:], sh[:], 1.0 / D, EPS, OP.mult, OP.add)
                nc.vector.tensor_scalar(b_[:], sv[:], 1.0 / D, EPS, OP.mult, OP.add)
                nc.vector.tensor_tensor(out=r_[:], in0=a_[:], in1=b_[:], op=OP.mult)
                nc.vector.reciprocal(rr[:], r_[:])
                nc.scalar.activation(s_[:], rr[:], AF.Sqrt)
                # graw = gd * (1/sqrt(D)) * s
                nc.vector.scalar_tensor_tensor(
                    out=gr[:], in0=gd[:], scalar=float(1.0 / np.sqrt(D)), in1=s_[:],
                    op0=OP.mult, op1=OP.mult,
                )
                # signed sqrt: pg = gr * rsqrt(max(|gr|, 1e-6))
                #   == sign(gr)*sqrt(|gr|) for |gr| >= 1e-6
                nc.vector.tensor_scalar(g1[:], gr[:], -1.0, None, OP.mult)
                nc.vector.tensor_tensor(out=g2[:], in0=gr[:], in1=g1[:], op=OP.max)
                nc.vector.tensor_scalar(ab[:], g2[:], 1e-6, None, OP.max)
                nc.vector.reciprocal(sg[:], ab[:])
                nc.scalar.activation(pq[:], sg[:], AF.Sqrt)
                nc.vector.tensor_tensor(out=pg[:], in0=gr[:], in1=pq[:], op=OP.mult)
                nc.scalar.activation(gate[:], pg[:], AF.Sigmoid)

                # out = gate * v
                nc.scalar.activation(
                    out_sb[:], v_sb[:], AF.Copy, scale=gate[:]
                )
                nc.sync.dma_start(out=out[i * P:(i + 1) * P, :], in_=out_sb[:])

            # ---- tiles 0 and 1: fused k-loop chasing the WvT chunk stream
            # (each chunk feeds 8 matmuls instead of 4, so the PE keeps up
            # with the DMA while WvT lands) ----
            v_ps0 = psV.tile([P, D], dt.float32, tag="vps")
            v_ps1 = psV.tile([P, D], dt.float32, tag="vps")
            for k in range(KCH):
                do_matmul(pending[0][1], v_ps0, k)
                do_matmul(pending[1][1], v_ps1, k)
            # prefetch 2..4 only after WvT is fully streamed
            pending.append(issue_loads(2, after=last_wvt))
            pending.append(issue_loads(3, after=last_wvt))
            epilogue(0, pending[0][0], v_ps0)
            pending.append(issue_loads(4, after=last_wvt))
            epilogue(1, pending[1][0], v_ps1)

            # ---- steady state ----
            for i in range(2, NTILES):
                h_sb, eT_sb = pending[i]
                if i + 3 < NTILES:
                    pending.append(issue_loads(i + 3))
                v_ps = psV.tile([P, D], dt.float32, tag="vps")
                for k in range(KCH):
                    do_matmul(eT_sb, v_ps, k)
                epilogue(i, h_sb, v_ps)

    nc.compile()
    return nc


def _get_program():
    global _PROGRAM
    if _PROGRAM is None:
        _PROGRAM = _build_program()
    return _PROGRAM


def _prep_inputs(hidden, tables2, tables3, Wv, gate_h_w, gate_v_w, input_ids):
    t2 = np.ascontiguousarray(
        np.asarray(tables2).transpose(1, 0, 2).reshape(TS, HE).astype(np.float16)
    )
    t3 = np.ascontiguousarray(
        np.asarray(tables3).transpose(1, 0, 2).reshape(TS, HE).astype(np.float16)
    )
    wvt16 = np.ascontiguousarray(np.asarray(Wv).T.astype(np.float16))
    wp = np.ascontiguousarray(
        np.broadcast_to(
            (np.asarray(gate_h_w) * np.asarray(gate_v_w)).astype(np.float32), (P, D)
        )
    )
    idx_full = _host_indices(np.asarray(input_ids))
    hid_full = np.asarray(hidden, dtype=np.float32).reshape(NT, D)

    in_maps = []
    for c in range(NCORES):
        sl = slice(c * TPC, (c + 1) * TPC)
        idx_c = np.ascontiguousarray(
            idx_full[sl].reshape(NTILES, P, 2).transpose(1, 0, 2).reshape(P, NTILES * 2)
        )
        in_maps.append({
            "tab2": t2,
            "tab3": t3,
            "wvt": wvt16,
            "wprod": wp,
            "idx": idx_c,
            "hid": np.ascontiguousarray(hid_full[sl]),
        })
    return in_maps


def kernel(hidden, tables2, tables3, Wv, gate_h_w, gate_v_w, input_ids, _trace=False):
    from concourse.bass_utils import run_bass_kernel_spmd

    nc = _get_program()
    in_maps = _prep_inputs(
        hidden, tables2, tables3, Wv, gate_h_w, gate_v_w, input_ids
    )
    res = run_bass_kernel_spmd(nc, in_maps, list(range(NCORES)), trace=_trace)
    out = np.concatenate([r["out"] for r in res.results], axis=0)
    kernel.last_results = res
    return out.reshape(B, T, D).astype(np.float32)


# revision 10
# speedup vs baseline: 1.1317x; 1.0144x over previous
"""EngramCache forward kernel for 8x Trainium2 NeuronCores (Bass/Tile).

Strategy: data-parallel over the 16384 tokens (2048 per core), hash tables
replicated. Host-side prep: n-gram hash indices (computed with jax-on-CPU to
bit-match the reference's int32 % lowering), tables merged to one
[8*TS, 256] fp16 table, Wv transposed to k-major fp16. Per core: one
indirect-DMA gather per 128-token tile -> PE transpose to k-major ->
fp16 matmul (fp32 PSUM accumulate) against SBUF-resident WvT -> fused
RMSNorm-gating epilogue on ACT/DVE.
"""

import numpy as np


def _ensure_import_paths():
    import sys
    for p in ("/opt/trn_rl_repo", "/root/.axon_site/_ro/trn_rl_repo"):
        if p not in sys.path:
            sys.path.insert(0, p)


_ensure_import_paths()

B, T, D, H, E = 4, 4096, 2048, 4, 256
TS, VOCAB = 100000, 50257
M0, M1, M2 = 20011, 30011, 40009
NCORES = 8
NT = B * T                 # 16384 tokens total
TPC = NT // NCORES         # 2048 tokens per core
P = 128
NTILES = TPC // P          # 16 token-tiles per core
HE = H * E                 # 1024: one table's concat-over-heads row
KCH = D // P               # 16 k-chunks of 128
DCH = 512                  # matmul moving free dim (one PSUM bank)
EPS = float(np.finfo(np.float32).eps)

_PROGRAM = None


def _host_indices(input_ids: np.ndarray) -> np.ndarray:
    """Merged-table row index per (token, segment): [NT, SEG] int32.

    Uses jax on CPU so the int32 `%` matches the reference bit-exactly
    (XLA-CPU lowers int32 division through a float path that yields
    negative remainders near the modulus; the reference clamps with max).
    """
    import jax
    import jax.numpy as jnp
    with jax.default_device(jax.devices("cpu")[0]):
        ids = jnp.clip(jnp.asarray(input_ids).astype(jnp.int32), 0, VOCAB - 1)
        s1 = jnp.pad(ids[:, :-1], ((0, 0), (1, 0)))
        s2 = jnp.pad(ids[:, :-2], ((0, 0), (2, 0)))
        hash2 = jnp.bitwise_xor(ids * M0, s1 * M1)
        hash3 = jnp.bitwise_xor(hash2, s2 * M2)
        i2 = np.asarray(jnp.maximum(hash2 % TS, 0)).reshape(-1)
        i3 = np.asarray(jnp.maximum(hash3 % TS, 0)).reshape(-1)
    idx = np.empty((NT, 2), np.int32)
    idx[:, 0] = i2
    idx[:, 1] = i3
    return idx


def _build_program():
    import concourse.bass as bass
    import concourse.bacc as bacc
    import concourse.mybir as mybir
    import concourse.tile as tile

    dt = mybir.dt
    nc = bacc.Bacc("TRN2", target_bir_lowering=False, debug=False)

    tab2 = nc.dram_tensor("tab2", [TS, HE], dt.float16, kind="ExternalInput")
    tab3 = nc.dram_tensor("tab3", [TS, HE], dt.float16, kind="ExternalInput")
    wvt = nc.dram_tensor("wvt", [D, D], dt.float16, kind="ExternalInput")
    wprod = nc.dram_tensor("wprod", [P, D], dt.float32, kind="ExternalInput")
    idx = nc.dram_tensor("idx", [P, NTILES * 2], dt.int32, kind="ExternalInput")
    hid = nc.dram_tensor("hid", [TPC, D], dt.float32, kind="ExternalInput")
    out = nc.dram_tensor("out", [TPC, D], dt.float32, kind="ExternalOutput")

    AF = mybir.ActivationFunctionType
    OP = mybir.AluOpType

    with tile.TileContext(nc) as tc:
        with (
            tc.tile_pool(name="persist", bufs=1) as persist,
            tc.tile_pool(name="eP", bufs=3) as e_pool,
            tc.tile_pool(name="etP", bufs=2) as et_pool,
            tc.tile_pool(name="hP", bufs=3) as h_pool,
            tc.tile_pool(name="oP", bufs=2) as o_pool,
            tc.tile_pool(name="vP", bufs=2) as v_pool,
            tc.tile_pool(name="wP", bufs=2) as vw_pool,
            tc.tile_pool(name="stats", bufs=2) as st_pool,
            tc.tile_pool(name="psV", bufs=2, space="PSUM") as psV,
        ):
            # ---- one-time loads ----
            # small tensors first so tile 0's gather/transpose aren't gated
            # behind the 16 MiB WvT stream
            idx_sb = persist.tile([P, NTILES * 2], dt.int32)
            nc.sync.dma_start(out=idx_sb[:], in_=idx[:])

            from concourse.tile_rust import add_dep_helper

            def issue_loads(i, after=None):
                """Gather + hidden load + xbar transpose for tile i."""
                e_sb = e_pool.tile([P, 2 * HE], dt.float16, tag="e")
                g1i = nc.gpsimd.indirect_dma_start(
                    out=e_sb[:, 0:HE],
                    out_offset=None,
                    in_=tab2[:],
                    in_offset=bass.IndirectOffsetOnAxis(
                        ap=idx_sb[:, 2 * i:2 * i + 1], axis=0
                    ),
                )
                g2i = nc.gpsimd.indirect_dma_start(
                    out=e_sb[:, HE:2 * HE],
                    out_offset=None,
                    in_=tab3[:],
                    in_offset=bass.IndirectOffsetOnAxis(
                        ap=idx_sb[:, 2 * i + 1:2 * i + 2], axis=0
                    ),
                )
                if after is not None:
                    # keep startup DMA bandwidth for the WvT stream
                    add_dep_helper(g1i.ins, after.ins,
                                   reason="throttle gather behind wvt")
                    add_dep_helper(g2i.ins, after.ins,
                                   reason="throttle gather behind wvt")
                h_sb = h_pool.tile([P, D], dt.float32, tag="h")
                nc.sync.dma_start(out=h_sb[:], in_=hid[i * P:(i + 1) * P, :])
                # out[p, c, t] = e[t, c*128 + p]: chunk-major k layout
                eT_sb = et_pool.tile([P, D], dt.float16, tag="eT")
                nc.sync.dma_start_transpose(
                    out=eT_sb[:].rearrange("p (c t) -> p c t", c=KCH),
                    in_=e_sb[:],
                )
                return h_sb, eT_sb

            def issue_gather(i):
                e_sb = e_pool.tile([P, 2 * HE], dt.float16, tag="e")
                nc.gpsimd.indirect_dma_start(
                    out=e_sb[:, 0:HE],
                    out_offset=None,
                    in_=tab2[:],
                    in_offset=bass.IndirectOffsetOnAxis(
                        ap=idx_sb[:, 2 * i:2 * i + 1], axis=0
                    ),
                )
                nc.gpsimd.indirect_dma_start(
                    out=e_sb[:, HE:2 * HE],
                    out_offset=None,
                    in_=tab3[:],
                    in_offset=bass.IndirectOffsetOnAxis(
                        ap=idx_sb[:, 2 * i + 1:2 * i + 2], axis=0
                    ),
                )
                return e_sb

            def issue_dmat(e_sb):
                eT_sb = et_pool.tile([P, D], dt.float16, tag="eT")
                nc.sync.dma_start_transpose(
                    out=eT_sb[:].rearrange("p (c t) -> p c t", c=KCH),
                    in_=e_sb[:],
                )
                return eT_sb

            # startup order: gathers 0/1 (SWDGE) -> their transposes (sync
            # ring, block only on the gathers) -> the 8 MiB WvT stream ->
            # wprod/hidden (not needed until the first epilogues)
            e0 = issue_gather(0)
            e1 = issue_gather(1)
            eT0 = issue_dmat(e0)
            eT1 = issue_dmat(e1)

            wvt_sb = persist.tile([P, KCH * D], dt.float16)   # 64KB/partition
            wvt_r = wvt[:].rearrange("(c p) d -> c p d", p=P)
            for c in range(KCH):
                last_wvt = nc.sync.dma_start(
                    out=wvt_sb[:, c * D:(c + 1) * D], in_=wvt_r[c]
                )

            wprod_sb = persist.tile([P, D], dt.float32)
            nc.sync.dma_start(out=wprod_sb[:], in_=wprod[:])

            def issue_hidden(i):
                h_sb = h_pool.tile([P, D], dt.float32, tag="h")
                nc.sync.dma_start(out=h_sb[:], in_=hid[i * P:(i + 1) * P, :])
                return h_sb

            h0 = issue_hidden(0)
            h1 = issue_hidden(1)
            pending = [(h0, eT0), (h1, eT1)]

            def do_matmul(eT_sb, v_ps, k):
                lhsT = eT_sb[:, k * P:(k + 1) * P]
                for dc in range(D // DCH):
                    nc.tensor.matmul(
                        v_ps[:, dc * DCH:(dc + 1) * DCH],
                        lhsT,
                        wvt_sb[:, k * D + dc * DCH: k * D + (dc + 1) * DCH],
                        start=(k == 0),
                        stop=(k == KCH - 1),
                    )

            def epilogue(i, h_sb, v_ps):
                # copy v out of PSUM first so the next tile's matmuls can
                # claim the bank immediately
                v_sb = v_pool.tile([P, D], dt.float32, tag="v")
                nc.scalar.copy(v_sb[:], v_ps[:])
                vw_sb = vw_pool.tile([P, D], dt.float32, tag="vw")
                out_sb = o_pool.tile([P, D], dt.float32, tag="o")

                sh = st_pool.tile([P, 1], dt.float32, tag="sh")
                sv = st_pool.tile([P, 1], dt.float32, tag="sv")
                gd = st_pool.tile([P, 1], dt.float32, tag="gd")

                # sum h^2 (junk full output into vw_sb, overwritten below)
                nc.scalar.activation(vw_sb[:], h_sb[:], AF.Square, accum_out=sh[:])
                # sum v^2 (junk full output into out_sb, overwritten below)
                nc.scalar.activation(out_sb[:], v_sb[:], AF.Square, accum_out=sv[:])
                # vw = v * wprod
                nc.vector.tensor_tensor(
                    out=vw_sb[:], in0=v_sb[:], in1=wprod_sb[:], op=OP.mult
                )
                # gd = sum(vw * h)
                nc.vector.scalar_tensor_tensor(
                    out=out_sb[:], in0=vw_sb[:], scalar=1.0, in1=h_sb[:],
                    op0=OP.mult, op1=OP.mult, accum_out=gd[:],
                )

                a_ = st_pool.tile([P, 1], dt.float32, tag="a")
                b_ = st_pool.tile([P, 1], dt.float32, tag="b")
                r_ = st_pool.tile([P, 1], dt.float32, tag="r")
                rr = st_pool.tile([P, 1], dt.float32, tag="rr")
                s_ = st_pool.tile([P, 1], dt.float32, tag="s")
                gr = st_pool.tile([P, 1], dt.float32, tag="gr")
                ab = st_pool.tile([P, 1], dt.float32, tag="ab")
                pq = st_pool.tile([P, 1], dt.float32, tag="pq")
                g1 = st_pool.tile([P, 1], dt.float32, tag="g1")
                g2 = st_pool.tile([P, 1], dt.float32, tag="g2")
                sg = st_pool.tile([P, 1], dt.float32, tag="sg")
                pg = st_pool.tile([P, 1], dt.float32, tag="pg")
                gate = st_pool.tile([P, 1], dt.float32, tag="gate")

                # a = sh/D + eps ; b = sv/D + eps ; r = a*b ; s = rsqrt(r)
                nc.vector.tensor_scalar(a_[:], sh[:], 1.0 / D, EPS, OP.mult, OP.add)
                nc.vector.tensor_scalar(b_[:], sv[:], 1.0 / D, EPS, OP.mult, OP.add)
                nc.vector.tensor_tensor(out=r_[:], in0=a_[:], in1=b_[:], op=OP.mult)
                nc.vector.reciprocal(rr[:], r_[:])
                nc.scalar.activation(s_[:], rr[:], AF.Sqrt)
                # graw = gd * (1/sqrt(D)) * s
                nc.vector.scalar_tensor_tensor(
                    out=gr[:], in0=gd[:], scalar=float(1.0 / np.sqrt(D)), in1=s_[:],
                    op0=OP.mult, op1=OP.mult,
                )
                # signed sqrt: pg = gr * rsqrt(max(|gr|, 1e-6))
                #   == sign(gr)*sqrt(|gr|) for |gr| >= 1e-6
                nc.vector.tensor_scalar(g1[:], gr[:], -1.0, None, OP.mult)
                nc.vector.tensor_tensor(out=g2[:], in0=gr[:], in1=g1[:], op=OP.max)
                nc.vector.tensor_scalar(ab[:], g2[:], 1e-6, None, OP.max)
                nc.vector.reciprocal(sg[:], ab[:])
                nc.scalar.activation(pq[:], sg[:], AF.Sqrt)
                nc.vector.tensor_tensor(out=pg[:], in0=gr[:], in1=pq[:], op=OP.mult)
                nc.scalar.activation(gate[:], pg[:], AF.Sigmoid)

                # out = gate * v
                nc.scalar.activation(
                    out_sb[:], v_sb[:], AF.Copy, scale=gate[:]
                )
                nc.sync.dma_start(out=out[i * P:(i + 1) * P, :], in_=out_sb[:])

            # ---- tiles 0 and 1: fused k-loop chasing the WvT chunk stream
            # (each chunk feeds 8 matmuls instead of 4, so the PE keeps up
            # with the DMA while WvT lands) ----
            v_ps0 = psV.tile([P, D], dt.float32, tag="vps")
            v_ps1 = psV.tile([P, D], dt.float32, tag="vps")
            for k in range(KCH):
                do_matmul(pending[0][1], v_ps0, k)
                do_matmul(pending[1][1], v_ps1, k)
            # prefetch 2..4 only after WvT is fully streamed
            pending.append(issue_loads(2, after=last_wvt))
            pending.append(issue_loads(3, after=last_wvt))
            epilogue(0, pending[0][0], v_ps0)
            pending.append(issue_loads(4, after=last_wvt))
            epilogue(1, pending[1][0], v_ps1)

            # ---- steady state ----
            for i in range(2, NTILES):
                h_sb, eT_sb = pending[i]
                if i + 3 < NTILES:
                    pending.append(issue_loads(i + 3))
                v_ps = psV.tile([P, D], dt.float32, tag="vps")
                for k in range(KCH):
                    do_matmul(eT_sb, v_ps, k)
                epilogue(i, h_sb, v_ps)

    nc.compile()
    return nc


def _get_program():
    global _PROGRAM
    if _PROGRAM is None:
        _PROGRAM = _build_program()
    return _PROGRAM


def _prep_inputs(hidden, tables2, tables3, Wv, gate_h_w, gate_v_w, input_ids):
    t2 = np.ascontiguousarray(
        np.asarray(tables2).transpose(1, 0, 2).reshape(TS, HE).astype(np.float16)
    )
    t3 = np.ascontiguousarray(
        np.asarray(tables3).transpose(1, 0, 2).reshape(TS, HE).astype(np.float16)
    )
    wvt16 = np.ascontiguousarray(np.asarray(Wv).T.astype(np.float16))
    wp = np.ascontiguousarray(
        np.broadcast_to(
            (np.asarray(gate_h_w) * np.asarray(gate_v_w)).astype(np.float32), (P, D)
        )
    )
    idx_full = _host_indices(np.asarray(input_ids))
    hid_full = np.asarray(hidden, dtype=np.float32).reshape(NT, D)

    in_maps = []
    for c in range(NCORES):
        sl = slice(c * TPC, (c + 1) * TPC)
        idx_c = np.ascontiguousarray(
            idx_full[sl].reshape(NTILES, P, 2).transpose(1, 0, 2).reshape(P, NTILES * 2)
        )
        in_maps.append({
            "tab2": t2,
            "tab3": t3,
            "wvt": wvt16,
            "wprod": wp,
            "idx": idx_c,
            "hid": np.ascontiguousarray(hid_full[sl]),
        })
    return in_maps


def kernel(hidden, tables2, tables3, Wv, gate_h_w, gate_v_w, input_ids, _trace=False):
    from concourse.bass_utils import run_bass_kernel_spmd

    nc = _get_program()
    in_maps = _prep_inputs(
        hidden, tables2, tables3, Wv, gate_h_w, gate_v_w, input_ids
    )
    res = run_bass_kernel_spmd(nc, in_maps, list(range(NCORES)), trace=_trace)
    out = np.concatenate([r["out"] for r in res.results], axis=0)
    kernel.last_results = res
    return out.reshape(B, T, D).astype(np.float32)


# revision 11
# speedup vs baseline: 1.1968x; 1.0575x over previous
"""EngramCache forward kernel for 8x Trainium2 NeuronCores (Bass/Tile).

Strategy: data-parallel over the 16384 tokens (2048 per core), hash tables
replicated. Host-side prep: n-gram hash indices (computed with jax-on-CPU to
bit-match the reference's int32 % lowering), tables merged to one
[8*TS, 256] fp16 table, Wv transposed to k-major fp16. Per core: one
indirect-DMA gather per 128-token tile -> PE transpose to k-major ->
fp16 matmul (fp32 PSUM accumulate) against SBUF-resident WvT -> fused
RMSNorm-gating epilogue on ACT/DVE.
"""

import numpy as np


def _ensure_import_paths():
    import sys
    for p in ("/opt/trn_rl_repo", "/root/.axon_site/_ro/trn_rl_repo"):
        if p not in sys.path:
            sys.path.insert(0, p)


_ensure_import_paths()

B, T, D, H, E = 4, 4096, 2048, 4, 256
TS, VOCAB = 100000, 50257
M0, M1, M2 = 20011, 30011, 40009
NCORES = 8
NT = B * T                 # 16384 tokens total
TPC = NT // NCORES         # 2048 tokens per core
P = 128
NTILES = TPC // P          # 16 token-tiles per core
HE = H * E                 # 1024: one table's concat-over-heads row
KCH = D // P               # 16 k-chunks of 128
DCH = 512                  # matmul moving free dim (one PSUM bank)
EPS = float(np.finfo(np.float32).eps)

_PROGRAM = None


def _host_indices(input_ids: np.ndarray) -> np.ndarray:
    """Merged-table row index per (token, segment): [NT, SEG] int32.

    Uses jax on CPU so the int32 `%` matches the reference bit-exactly
    (XLA-CPU lowers int32 division through a float path that yields
    negative remainders near the modulus; the reference clamps with max).
    """
    import jax
    import jax.numpy as jnp
    with jax.default_device(jax.devices("cpu")[0]):
        ids = jnp.clip(jnp.asarray(input_ids).astype(jnp.int32), 0, VOCAB - 1)
        s1 = jnp.pad(ids[:, :-1], ((0, 0), (1, 0)))
        s2 = jnp.pad(ids[:, :-2], ((0, 0), (2, 0)))
        hash2 = jnp.bitwise_xor(ids * M0, s1 * M1)
        hash3 = jnp.bitwise_xor(hash2, s2 * M2)
        i2 = np.asarray(jnp.maximum(hash2 % TS, 0)).reshape(-1)
        i3 = np.asarray(jnp.maximum(hash3 % TS, 0)).reshape(-1)
    idx = np.empty((NT, 2), np.int32)
    idx[:, 0] = i2
    idx[:, 1] = i3
    return idx


def _build_program():
    import concourse.bass as bass
    import concourse.bacc as bacc
    import concourse.mybir as mybir
    import concourse.tile as tile

    dt = mybir.dt
    nc = bacc.Bacc("TRN2", target_bir_lowering=False, debug=False)

    tab2 = nc.dram_tensor("tab2", [TS, HE], dt.float16, kind="ExternalInput")
    tab3 = nc.dram_tensor("tab3", [TS, HE], dt.float16, kind="ExternalInput")
    wvt = nc.dram_tensor("wvt", [D, D], dt.float16, kind="ExternalInput")
    wprod = nc.dram_tensor("wprod", [P, D], dt.float32, kind="ExternalInput")
    idx = nc.dram_tensor("idx", [P, NTILES * 2], dt.int32, kind="ExternalInput")
    hid = nc.dram_tensor("hid", [TPC, D], dt.float32, kind="ExternalInput")
    out = nc.dram_tensor("out", [TPC, D], dt.float32, kind="ExternalOutput")

    AF = mybir.ActivationFunctionType
    OP = mybir.AluOpType

    with tile.TileContext(nc) as tc:
        with (
            tc.tile_pool(name="persist", bufs=1) as persist,
            tc.tile_pool(name="eP", bufs=3) as e_pool,
            tc.tile_pool(name="etP", bufs=2) as et_pool,
            tc.tile_pool(name="hP", bufs=3) as h_pool,
            tc.tile_pool(name="oP", bufs=2) as o_pool,
            tc.tile_pool(name="vP", bufs=2) as v_pool,
            tc.tile_pool(name="wP", bufs=2) as vw_pool,
            tc.tile_pool(name="stats", bufs=2) as st_pool,
            tc.tile_pool(name="psV", bufs=2, space="PSUM") as psV,
        ):
            # ---- one-time loads ----
            # small tensors first so tile 0's gather/transpose aren't gated
            # behind the 16 MiB WvT stream
            idx_sb = persist.tile([P, NTILES * 2], dt.int32)
            nc.sync.dma_start(out=idx_sb[:], in_=idx[:])

            from concourse.tile_rust import add_dep_helper

            def issue_loads(i, after=None):
                """Gather + hidden load + xbar transpose for tile i."""
                e_sb = e_pool.tile([P, 2 * HE], dt.float16, tag="e")
                g1i = nc.gpsimd.indirect_dma_start(
                    out=e_sb[:, 0:HE],
                    out_offset=None,
                    in_=tab2[:],
                    in_offset=bass.IndirectOffsetOnAxis(
                        ap=idx_sb[:, 2 * i:2 * i + 1], axis=0
                    ),
                )
                g2i = nc.gpsimd.indirect_dma_start(
                    out=e_sb[:, HE:2 * HE],
                    out_offset=None,
                    in_=tab3[:],
                    in_offset=bass.IndirectOffsetOnAxis(
                        ap=idx_sb[:, 2 * i + 1:2 * i + 2], axis=0
                    ),
                )
                if after is not None:
                    # keep startup DMA bandwidth for the WvT stream
                    add_dep_helper(g1i.ins, after.ins,
                                   reason="throttle gather behind wvt")
                    add_dep_helper(g2i.ins, after.ins,
                                   reason="throttle gather behind wvt")
                h_sb = h_pool.tile([P, D], dt.float32, tag="h")
                nc.sync.dma_start(out=h_sb[:], in_=hid[i * P:(i + 1) * P, :])
                # out[p, c, t] = e[t, c*128 + p]: chunk-major k layout
                eT_sb = et_pool.tile([P, D], dt.float16, tag="eT")
                nc.sync.dma_start_transpose(
                    out=eT_sb[:].rearrange("p (c t) -> p c t", c=KCH),
                    in_=e_sb[:],
                )
                return h_sb, eT_sb

            def issue_gather(i):
                e_sb = e_pool.tile([P, 2 * HE], dt.float16, tag="e")
                nc.gpsimd.indirect_dma_start(
                    out=e_sb[:, 0:HE],
                    out_offset=None,
                    in_=tab2[:],
                    in_offset=bass.IndirectOffsetOnAxis(
                        ap=idx_sb[:, 2 * i:2 * i + 1], axis=0
                    ),
                )
                nc.gpsimd.indirect_dma_start(
                    out=e_sb[:, HE:2 * HE],
                    out_offset=None,
                    in_=tab3[:],
                    in_offset=bass.IndirectOffsetOnAxis(
                        ap=idx_sb[:, 2 * i + 1:2 * i + 2], axis=0
                    ),
                )
                return e_sb

            def issue_dmat(e_sb):
                eT_sb = et_pool.tile([P, D], dt.float16, tag="eT")
                nc.sync.dma_start_transpose(
                    out=eT_sb[:].rearrange("p (c t) -> p c t", c=KCH),
                    in_=e_sb[:],
                )
                return eT_sb

            # startup order: gathers 0/1 (SWDGE) -> their transposes (sync
            # ring, block only on the gathers) -> the 8 MiB WvT stream ->
            # wprod/hidden (not needed until the first epilogues)
            e0 = issue_gather(0)
            e1 = issue_gather(1)
            eT0 = issue_dmat(e0)
            eT1 = issue_dmat(e1)

            # 16 separate tiles so each matmul k-chunk depends only on its
            # own 512KB DMA, not the whole 8 MiB stream
            wvt_r = wvt[:].rearrange("(c p) d -> c p d", p=P)
            wvt_tiles = []
            for c in range(KCH):
                wt = persist.tile([P, D], dt.float16, tag=f"wvt{c}")
                last_wvt = nc.sync.dma_start(out=wt[:], in_=wvt_r[c])
                wvt_tiles.append(wt)

            wprod_sb = persist.tile([P, D], dt.float32)
            nc.sync.dma_start(out=wprod_sb[:], in_=wprod[:])

            def issue_hidden(i):
                h_sb = h_pool.tile([P, D], dt.float32, tag="h")
                nc.sync.dma_start(out=h_sb[:], in_=hid[i * P:(i + 1) * P, :])
                return h_sb

            h0 = issue_hidden(0)
            h1 = issue_hidden(1)
            pending = [(h0, eT0), (h1, eT1)]

            def do_matmul(eT_sb, v_ps, k):
                lhsT = eT_sb[:, k * P:(k + 1) * P]
                for dc in range(D // DCH):
                    nc.tensor.matmul(
                        v_ps[:, dc * DCH:(dc + 1) * DCH],
                        lhsT,
                        wvt_tiles[k][:, dc * DCH:(dc + 1) * DCH],
                        start=(k == 0),
                        stop=(k == KCH - 1),
                    )

            def epilogue(i, h_sb, v_ps):
                # copy v out of PSUM first so the next tile's matmuls can
                # claim the bank immediately
                v_sb = v_pool.tile([P, D], dt.float32, tag="v")
                nc.scalar.copy(v_sb[:], v_ps[:])
                vw_sb = vw_pool.tile([P, D], dt.float32, tag="vw")
                out_sb = o_pool.tile([P, D], dt.float32, tag="o")

                sh = st_pool.tile([P, 1], dt.float32, tag="sh")
                sv = st_pool.tile([P, 1], dt.float32, tag="sv")
                gd = st_pool.tile([P, 1], dt.float32, tag="gd")

                # sum h^2 (junk full output into vw_sb, overwritten below)
                nc.scalar.activation(vw_sb[:], h_sb[:], AF.Square, accum_out=sh[:])
                # sum v^2 (junk full output into out_sb, overwritten below)
                nc.scalar.activation(out_sb[:], v_sb[:], AF.Square, accum_out=sv[:])
                # vw = v * wprod -- read v from PSUM so this starts right
                # after the matmuls, in parallel with the ACT copy
                nc.vector.tensor_tensor(
                    out=vw_sb[:], in0=v_ps[:], in1=wprod_sb[:], op=OP.mult
                )
                # gd = sum(vw * h)
                nc.vector.scalar_tensor_tensor(
                    out=out_sb[:], in0=vw_sb[:], scalar=1.0, in1=h_sb[:],
                    op0=OP.mult, op1=OP.mult, accum_out=gd[:],
                )

                a_ = st_pool.tile([P, 1], dt.float32, tag="a")
                b_ = st_pool.tile([P, 1], dt.float32, tag="b")
                r_ = st_pool.tile([P, 1], dt.float32, tag="r")
                rr = st_pool.tile([P, 1], dt.float32, tag="rr")
                s_ = st_pool.tile([P, 1], dt.float32, tag="s")
                gr = st_pool.tile([P, 1], dt.float32, tag="gr")
                ab = st_pool.tile([P, 1], dt.float32, tag="ab")
                pq = st_pool.tile([P, 1], dt.float32, tag="pq")
                g1 = st_pool.tile([P, 1], dt.float32, tag="g1")
                g2 = st_pool.tile([P, 1], dt.float32, tag="g2")
                sg = st_pool.tile([P, 1], dt.float32, tag="sg")
                pg = st_pool.tile([P, 1], dt.float32, tag="pg")
                gate = st_pool.tile([P, 1], dt.float32, tag="gate")

                # a = sh/D + eps ; b = sv/D + eps ; r = a*b ; s = rsqrt(r)
                nc.vector.tensor_scalar(a_[:], sh[:], 1.0 / D, EPS, OP.mult, OP.add)
                nc.vector.tensor_scalar(b_[:], sv[:], 1.0 / D, EPS, OP.mult, OP.add)
                nc.vector.tensor_tensor(out=r_[:], in0=a_[:], in1=b_[:], op=OP.mult)
                nc.vector.reciprocal(rr[:], r_[:])
                nc.scalar.activation(s_[:], rr[:], AF.Sqrt)
                # graw = gd * (1/sqrt(D)) * s
                nc.vector.scalar_tensor_tensor(
                    out=gr[:], in0=gd[:], scalar=float(1.0 / np.sqrt(D)), in1=s_[:],
                    op0=OP.mult, op1=OP.mult,
                )
                # signed sqrt: pg = gr * rsqrt(max(|gr|, 1e-6))
                #   == sign(gr)*sqrt(|gr|) for |gr| >= 1e-6
                nc.vector.tensor_scalar(g1[:], gr[:], -1.0, None, OP.mult)
                nc.vector.tensor_tensor(out=g2[:], in0=gr[:], in1=g1[:], op=OP.max)
                nc.vector.tensor_scalar(ab[:], g2[:], 1e-6, None, OP.max)
                nc.vector.reciprocal(sg[:], ab[:])
                nc.scalar.activation(pq[:], sg[:], AF.Sqrt)
                nc.vector.tensor_tensor(out=pg[:], in0=gr[:], in1=pq[:], op=OP.mult)
                nc.scalar.activation(gate[:], pg[:], AF.Sigmoid)

                # out = gate * v  (DVE tensor_scalar, fp32 SBUF 2x mode)
                nc.vector.tensor_scalar(out_sb[:], v_sb[:], gate[:], None, OP.mult)
                nc.sync.dma_start(out=out[i * P:(i + 1) * P, :], in_=out_sb[:])

            for i in range(NTILES):
                h_sb, eT_sb = pending[i]
                if i + 2 < NTILES:
                    # tiles 2-4's gathers wait for the WvT stream to finish
                    # so it keeps full DMA bandwidth during startup
                    pending.append(
                        issue_loads(i + 2, after=last_wvt if i + 2 <= 4 else None)
                    )
                v_ps = psV.tile([P, D], dt.float32, tag="vps")
                for k in range(KCH):
                    do_matmul(eT_sb, v_ps, k)
                epilogue(i, h_sb, v_ps)

    nc.compile()
    return nc


def _get_program():
    global _PROGRAM
    if _PROGRAM is None:
        _PROGRAM = _build_program()
    return _PROGRAM


def _prep_inputs(hidden, tables2, tables3, Wv, gate_h_w, gate_v_w, input_ids):
    t2 = np.ascontiguousarray(
        np.asarray(tables2).transpose(1, 0, 2).reshape(TS, HE).astype(np.float16)
    )
    t3 = np.ascontiguousarray(
        np.asarray(tables3).transpose(1, 0, 2).reshape(TS, HE).astype(np.float16)
    )
    wvt16 = np.ascontiguousarray(np.asarray(Wv).T.astype(np.float16))
    wp = np.ascontiguousarray(
        np.broadcast_to(
            (np.asarray(gate_h_w) * np.asarray(gate_v_w)).astype(np.float32), (P, D)
        )
    )
    idx_full = _host_indices(np.asarray(input_ids))
    hid_full = np.asarray(hidden, dtype=np.float32).reshape(NT, D)

    in_maps = []
    for c in range(NCORES):
        sl = slice(c * TPC, (c + 1) * TPC)
        idx_c = np.ascontiguousarray(
            idx_full[sl].reshape(NTILES, P, 2).transpose(1, 0, 2).reshape(P, NTILES * 2)
        )
        in_maps.append({
            "tab2": t2,
            "tab3": t3,
            "wvt": wvt16,
            "wprod": wp,
            "idx": idx_c,
            "hid": np.ascontiguousarray(hid_full[sl]),
        })
    return in_maps


def kernel(hidden, tables2, tables3, Wv, gate_h_w, gate_v_w, input_ids, _trace=False):
    from concourse.bass_utils import run_bass_kernel_spmd

    nc = _get_program()
    in_maps = _prep_inputs(
        hidden, tables2, tables3, Wv, gate_h_w, gate_v_w, input_ids
    )
    res = run_bass_kernel_spmd(nc, in_maps, list(range(NCORES)), trace=_trace)
    out = np.concatenate([r["out"] for r in res.results], axis=0)
    kernel.last_results = res
    return out.reshape(B, T, D).astype(np.float32)
